# revision 7
# baseline (speedup 1.0000x reference)
"""Trainium2 Bass kernel for the AttentionBlock problem.

Reference computation (per batch n):
    sim[c, d]  = sum_s K[c, s] * Q[d, s] / sqrt(C)
    sim'       = softmax(sim, axis=c)
    out[c, s]  = sum_d sim'[c, d] * V[d, s]

Strategy: pure data parallel over the batch dim N=16 across 8 NeuronCores
(2 batches per core).  Per batch, on-chip:
    simT[d, c] = sum_s Q[d,s] K[c,s]   (d on partitions -> softmax along the
                                        free axis c; PE-transpose Q,K chunks
                                        to get the s-major operands)
    E[d, c]    = exp(scale*simT - scale*max_c) / sum  (ScalarE exp with fused
                                        row-max bias and fused row-sum)
    out[c, s]  = sum_d E[d, c] V[d,s]  (E is directly the lhsT; V natural
                                        layout is directly the rhs)
Matmuls run as float32r (TF32-like: full fp32 storage, 1 cycle/row at
free width >= 256).

The kernel is DMA-bound: 58.7 MB/core/iteration at ~360 GB/s ~= 163 us,
with PE busy ~= 160 us just underneath.  The emission is therefore fully
software-pipelined:
  - bf16 output write (halves output traffic; ~3e-3 rel err, gate is 2e-2)
  - phase C streams V column-chunks per sj instead of keeping V resident
  - v loads + output writes ride the gpsimd (SWDGE) queue so they never
    head-of-line-block q/k loads on SP
  - phase C of batch b is emitted INTERLEAVED with phase A of batch b+1:
    the PE queue alternates [ctx-group, transpose-group, sim-matmul-group],
    so ctx matmuls fill the transpose->PSUM-copy->matmul latency that
    otherwise stalls PE ~1.8 us per group (and drops it to mid p-state)
  - sim matmuls lag their transpose group by one (pend), giving the
    PSUM->SBUF copies a full group of PE work to drain under
  - PSUM: 4 sim banks + 2 transpose banks + 2 ctx banks
"""
import sys

sys.path.insert(0, "/opt/trn_rl_repo")
sys.path.insert(0, "/root/.axon_site")

import numpy as np

N, C, S = 16, 512, 4096
N_CORES = 8
B = N // N_CORES          # batches per core
P = 128
CT = C // P               # 4 partition tiles over C
DMA_W = 1024              # q/k dma chunk width (free dim)
NDMA = S // DMA_W         # 4 dma chunks per tensor per batch
TPW = DMA_W // P          # 8 transpose sub-chunks per dma chunk
MMW = 512                 # context matmul free width
NMM = S // MMW            # 8 context free chunks
NG = S // P               # 32 groups: transpose-groups == ctx-groups

_CACHE = {}


def _emit_step(nc, pools, ident, dram, cur, prev):
    """Emit phases A+B for batch `cur` interleaved with phase C for `prev`.

    cur:  (rep, b) or None (epilogue: only phase C of prev)
    prev: (rep, b, e_tiles) or None (prologue: only phases A+B of cur)
    Returns e_tiles for cur (or None).
    """
    import concourse.bass as bass
    from concourse import mybir

    f32 = mybir.dt.float32
    f32r = mybir.dt.float32r
    ts = bass.ts
    X = mybir.AxisListType.X
    EXP = mybir.ActivationFunctionType.Exp
    SCALE = float(C) ** -0.5

    (nat_pool, tsb_pool, v_pool, e_pool, small_pool, out_pool,
     tp_psum, sim_psum, ctx_psum) = pools
    q_d, k_d, v_d, o_d, out_dt = dram

    do_a = cur is not None
    do_c = prev is not None

    if do_a:
        rep, b = cur
        q_t = q_d.ap()[b].rearrange("(o p) s -> p o s", p=P)
        k_t = k_d.ap()[b].rearrange("(o p) s -> p o s", p=P)
        sim_ps = [
            sim_psum.tile([P, C], f32, tag="sim", name=f"sim_{rep}_{b}_{dt}")
            for dt in range(CT)
        ]
    if do_c:
        _prep, pb, e_prev = prev
        v_t = v_d.ap()[pb].rearrange("(o p) s -> p o s", p=P)
        o_t = o_d.ap()[pb].rearrange("(o p) s -> p o s", p=P)

    def mm(pend):
        qt, kt, j = pend
        for dt in range(CT):
            nc.tensor.matmul(
                sim_ps[dt][:], qt[:, ts(dt, P)], kt[:],
                start=(j == 0), stop=(j == NG - 1))

    pend = None
    qn = kn = vj = osj = None
    for i in range(NG):
        # -- DMA issues for this group --
        if do_a and i % TPW == 0:
            jj = i // TPW
            qn = nat_pool.tile([P, CT, DMA_W], f32r, tag="qnat")
            nc.sync.dma_start(qn[:], q_t[:, :, ts(jj, DMA_W)].bitcast(f32r))
            kn = nat_pool.tile([P, CT, DMA_W], f32r, tag="knat")
            nc.sync.dma_start(kn[:], k_t[:, :, ts(jj, DMA_W)].bitcast(f32r))
        if do_c and i % CT == 0:
            sj = i // CT
            vj = v_pool.tile([P, CT, MMW], f32r, tag="v")
            nc.gpsimd.dma_start(vj[:], v_t[:, :, ts(sj, MMW)].bitcast(f32r))
            osj = out_pool.tile([P, CT, MMW], out_dt, tag="ob")

        # -- phase C group for prev: one [128, MMW] ctx chunk --
        if do_c:
            sj, ct = i // CT, i % CT
            ctx = ctx_psum.tile([P, MMW], f32, tag="ctx")
            for dt in range(CT):
                nc.tensor.matmul(
                    ctx[:], e_prev[dt][:, ts(ct, P)], vj[:, dt, :],
                    start=(dt == 0), stop=(dt == CT - 1))
            if ct % 2 == 0:
                nc.vector.tensor_copy(osj[:, ct, :], ctx[:])
            else:
                nc.scalar.copy(osj[:, ct, :], ctx[:])
            if ct == CT - 1:
                nc.gpsimd.dma_start(o_t[:, :, ts(sj, MMW)], osj[:])

        # -- phase A transpose group for cur --
        if do_a:
            j4 = i % TPW
            qt_ps = tp_psum.tile([P, C], f32r, tag="tp")
            for o in range(CT):
                nc.tensor.transpose(
                    qt_ps[:, ts(o, P)], qn[:, o, ts(j4, P)], ident[:])
            kt_ps = tp_psum.tile([P, C], f32r, tag="tp")
            for o in range(CT):
                nc.tensor.transpose(
                    kt_ps[:, ts(o, P)], kn[:, o, ts(j4, P)], ident[:])
            qt = tsb_pool.tile([P, C], f32r, tag="qt")
            nc.vector.tensor_copy(qt[:], qt_ps[:])
            kt = tsb_pool.tile([P, C], f32r, tag="kt")
            nc.scalar.copy(kt[:], kt_ps[:])
            if pend is not None:
                mm(pend)
            pend = (qt, kt, i)

    if not do_a:
        return None
    mm(pend)

    # ---- phase B: row softmax along the free axis ----
    e_tiles = []
    for dt in range(CT):
        neg_mx = small_pool.tile([P, 1], f32, tag="nmx")
        nc.vector.reduce_max(neg_mx[:], sim_ps[dt][:], axis=X, negate=True)
        nmx_s = small_pool.tile([P, 1], f32, tag="nmxs")
        nc.vector.tensor_scalar_mul(nmx_s[:], neg_mx[:], SCALE)
        e32 = tsb_pool.tile([P, C], f32, tag="e32")
        ssum = small_pool.tile([P, 1], f32, tag="ssum")
        nc.scalar.activation(
            e32[:], sim_ps[dt][:], EXP,
            bias=nmx_s[:], scale=SCALE, accum_out=ssum[:])
        rr = small_pool.tile([P, 1], f32, tag="rr")
        nc.vector.reciprocal(rr[:], ssum[:])
        e_sb = e_pool.tile([P, C], f32r, tag="e")
        nc.vector.tensor_scalar_mul(e_sb[:], e32[:], rr[:])
        e_tiles.append(e_sb)
    return e_tiles


def _build(reps=1, out_bf16=True):
    import concourse.bass as bass
    import concourse.tile as tile
    from concourse import bacc, mybir
    from concourse.masks import make_identity

    f32 = mybir.dt.float32
    out_dt = mybir.dt.bfloat16 if out_bf16 else f32

    nc = bacc.Bacc("TRN2", target_bir_lowering=False, debug=False,
                   num_devices=N_CORES)
    q_d = nc.dram_tensor("query", [B, C, S], f32, kind="ExternalInput")
    k_d = nc.dram_tensor("key", [B, C, S], f32, kind="ExternalInput")
    v_d = nc.dram_tensor("value", [B, C, S], f32, kind="ExternalInput")
    o_d = nc.dram_tensor("out", [B, C, S], out_dt, kind="ExternalOutput")
    dram = (q_d, k_d, v_d, o_d, out_dt)

    with tile.TileContext(nc) as tc:
        with (
            tc.tile_pool(name="const", bufs=1) as const_pool,
            tc.tile_pool(name="nat", bufs=2) as nat_pool,
            tc.tile_pool(name="tsb", bufs=4) as tsb_pool,
            tc.tile_pool(name="vpool", bufs=4) as v_pool,
            tc.tile_pool(name="epool", bufs=2 * CT) as e_pool,
            tc.tile_pool(name="small", bufs=8) as small_pool,
            tc.tile_pool(name="outp", bufs=3) as out_pool,
            tc.tile_pool(name="tp_ps", bufs=2, space="PSUM") as tp_psum,
            tc.tile_pool(name="sim_ps", bufs=CT, space="PSUM") as sim_psum,
            tc.tile_pool(name="ctx_ps", bufs=2, space="PSUM") as ctx_psum,
        ):
            ident32 = const_pool.tile([P, P], f32)
            make_identity(nc, ident32)
            ident = const_pool.tile([P, P], mybir.dt.float32r)
            nc.vector.tensor_copy(ident[:], ident32[:])

            pools = (nat_pool, tsb_pool, v_pool, e_pool, small_pool,
                     out_pool, tp_psum, sim_psum, ctx_psum)
            batches = [(rep, b) for rep in range(reps) for b in range(B)]
            prev = None
            for cur in batches:
                e_tiles = _emit_step(nc, pools, ident, dram, cur, prev)
                prev = (cur[0], cur[1], e_tiles)
            _emit_step(nc, pools, ident, dram, None, prev)

    nc.compile()
    return nc


def _get_nc(reps=1, out_bf16=True):
    key = (reps, out_bf16)
    if key not in _CACHE:
        _CACHE[key] = _build(reps, out_bf16)
    return _CACHE[key]


def run_sharded(inputs, trace=False, reps=1, out_bf16=True, **kwargs):
    """Run the SPMD kernel: returns (full_output_fp32, BassKernelResults)."""
    from concourse.bass_utils import run_bass_kernel_spmd

    nc = _get_nc(reps, out_bf16)
    in_maps = []
    for i in range(N_CORES):
        sl = slice(i * B, (i + 1) * B)
        in_maps.append({
            "query": np.ascontiguousarray(inputs["query"][sl]),
            "key": np.ascontiguousarray(inputs["key"][sl]),
            "value": np.ascontiguousarray(inputs["value"][sl]),
        })
    res = run_bass_kernel_spmd(
        nc, in_maps, core_ids=list(range(N_CORES)), trace=trace, **kwargs)
    out = np.concatenate(
        [np.asarray(res.results[i]["out"]).astype(np.float32)
         for i in range(N_CORES)], axis=0)
    return out, res


def kernel(**inputs):
    inputs = {k: np.asarray(v, dtype=np.float32) for k, v in inputs.items()}
    out, _ = run_sharded(inputs, trace=False)
    return out


# revision 32
# speedup vs baseline: 2.3614x; 2.3614x over previous
"""Trainium2 Bass kernel for the AttentionBlock problem.

Reference computation (per batch n):
    sim[c, d]  = sum_s K[c, s] * Q[d, s] / sqrt(C)
    sim'       = softmax(sim, axis=c)
    out[c, s]  = sum_d sim'[c, d] * V[d, s]

Strategy: pure data parallel over the batch dim N=16 across 8 NeuronCores
(2 batches per core).  Per batch, on-chip:
    simT[d, c] = sum_s Q[d,s] K[c,s]   (d on partitions -> softmax along the
                                        free axis c; PE-transpose Q,K chunks
                                        to get the s-major operands)
    E[d, c]    = exp(scale*simT - scale*max_c) / sum  (ScalarE exp with fused
                                        row-max bias and fused row-sum)
    out[c, s]  = sum_d E[d, c] V[d,s]  (E is directly the lhsT; V natural
                                        layout is directly the rhs)
Matmuls run as float32r (TF32-like: full fp32 storage, 1 cycle/row at
free width >= 256).

The kernel is DMA-bound: 58.7 MB/core/iteration at ~360 GB/s ~= 163 us,
with PE busy ~= 160 us just underneath.  The emission is therefore fully
software-pipelined:
  - bf16 output write (halves output traffic; ~3e-3 rel err, gate is 2e-2)
  - phase C streams V column-chunks per sj instead of keeping V resident
  - v loads + output writes ride the gpsimd (SWDGE) queue so they never
    head-of-line-block q/k loads on SP
  - phase C of batch b is emitted INTERLEAVED with phase A of batch b+1:
    the PE queue alternates [ctx-group, transpose-group, sim-matmul-group],
    so ctx matmuls fill the transpose->PSUM-copy->matmul latency that
    otherwise stalls PE ~1.8 us per group (and drops it to mid p-state)
  - sim matmuls lag their transpose group by one (pend), giving the
    PSUM->SBUF copies a full group of PE work to drain under
  - PSUM: 4 sim banks + 2 transpose banks + 2 ctx banks
"""
import sys

sys.path.insert(0, "/opt/trn_rl_repo")
sys.path.insert(0, "/root/.axon_site")

import numpy as np

N, C, S = 16, 512, 4096
N_CORES = 8
B = N // N_CORES          # batches per core
P = 128
CT = C // P               # 4 partition tiles over C
# q/k load chunk widths (must sum to S).  Uniform 1024 measured best in the
# timeline sim: splitting the first chunk smaller adds DMA dispatches that
# cost more than the earlier PE start saves.
CHUNKS = [1024, 1024, 1024, 1024]
CHUNK_OFF = [0, 1024, 2048, 3072]
MMW = 512                 # context matmul free width
NMM = S // MMW            # 8 context free chunks
NG = S // P               # 32 groups: transpose-groups == ctx-groups
# group index -> (chunk idx, sub-offset within chunk)
_G2C = []
for _ci, (_off, _w) in enumerate(zip(CHUNK_OFF, CHUNKS)):
    for _j4 in range(_w // P):
        _G2C.append((_ci, _j4))
assert len(_G2C) == NG

_CACHE = {}


def _emit_step(nc, pools, ident, dram, cur, prev):
    """Emit phases A+B for batch `cur` interleaved with phase C for `prev`.

    cur:  (rep, b) or None (epilogue: only phase C of prev)
    prev: (rep, b, e_tiles) or None (prologue: only phases A+B of cur)
    Returns e_tiles for cur (or None).
    """
    import concourse.bass as bass
    from concourse import mybir

    f32 = mybir.dt.float32
    f32r = mybir.dt.float32r
    ts = bass.ts
    X = mybir.AxisListType.X
    EXP = mybir.ActivationFunctionType.Exp
    SCALE = float(C) ** -0.5

    (nat_pool, tsb_pool, v_pool, e_pool, small_pool, out_pool,
     tp_psum, sim_psum, ctx_psum) = pools
    q_d, k_d, v_d, o_d, out_dt = dram

    do_a = cur is not None
    do_c = prev is not None

    if do_a:
        rep, b = cur
        q_t = q_d.ap()[b].rearrange("(o p) s -> p o s", p=P)
        k_t = k_d.ap()[b].rearrange("(o p) s -> p o s", p=P)
        sim_ps = [
            sim_psum.tile([P, C], f32, tag="sim", name=f"sim_{rep}_{b}_{dt}")
            for dt in range(CT)
        ]
    if do_c:
        _prep, pb, e_prev = prev
        v_t = v_d.ap()[pb].rearrange("(o p) s -> p o s", p=P)
        o_t = o_d.ap()[pb].rearrange("(o p) s -> p o s", p=P)

    def mm(pend):
        qt, kt, j = pend
        for dt in range(CT):
            nc.tensor.matmul(
                sim_ps[dt][:], qt[:, ts(dt, P)], kt[:],
                start=(j == 0), stop=(j == NG - 1))

    bf16 = mybir.dt.bfloat16
    ident, ident_bf = ident if isinstance(ident, tuple) else (ident, None)

    pend = []
    qn = kn = vj = osj = None
    for i in range(NG):
        # -- DMA issues for this group --
        if do_a and _G2C[i][1] == 0:
            ci = _G2C[i][0]
            off, w = CHUNK_OFF[ci], CHUNKS[ci]
            # q/k load through the gpsimd SWDGE queue with an in-flight
            # f32 -> bf16 cast (only SWDGE can cast): the PE transposes then
            # run at 1.0 cycle/row instead of f32r's 1.5, with no engine
            # cast cost, and the natural chunks take half the SBUF.
            qn = nat_pool.tile([P, CT, w], bf16, tag="qnat")
            nc.gpsimd.dma_start(qn[:], q_t[:, :, off:off + w])
            kn = nat_pool.tile([P, CT, w], bf16, tag="knat")
            nc.gpsimd.dma_start(kn[:], k_t[:, :, off:off + w])
        if do_c and i % CT == 0:
            sj = i // CT
            vj = v_pool.tile([P, CT, MMW], f32r, tag="v")
            nc.sync.dma_start(vj[:], v_t[:, :, ts(sj, MMW)].bitcast(f32r))
            osj = out_pool.tile([P, CT, MMW], out_dt, tag="ob")

        # -- phase C group for prev: one [128, MMW] ctx chunk --
        if do_c:
            sj, ct = i // CT, i % CT
            ctx = ctx_psum.tile([P, MMW], f32, tag="ctx")
            for dt in range(CT):
                nc.tensor.matmul(
                    ctx[:], e_prev[dt][:, ts(ct, P)], vj[:, dt, :],
                    start=(dt == 0), stop=(dt == CT - 1))
            if ct % 2 == 0:
                nc.vector.tensor_copy(osj[:, ct, :], ctx[:])
            else:
                nc.scalar.copy(osj[:, ct, :], ctx[:])
            if ct == CT - 1:
                nc.scalar.dma_start(o_t[:, :, ts(sj, MMW)], osj[:])

        # -- phase A transpose group for cur --
        if do_a:
            j4 = _G2C[i][1]
            # Prologue (no phase C underneath): the ctx banks are idle, so
            # alternate transpose groups between the tp and ctx bank pairs.
            # The 4-bank rotation removes the copy-latency stall that
            # otherwise paces the first batch (and parks PE at mid p-state).
            if do_c or i % 2 == 0:
                tp_p, tp_tag = tp_psum, "tp"
            else:
                tp_p, tp_tag = ctx_psum, "ctx"
            qt_ps = tp_p.tile([P, C], bf16, tag=tp_tag)
            for o in range(CT):
                nc.tensor.transpose(
                    qt_ps[:, ts(o, P)], qn[:, o, ts(j4, P)], ident_bf[:])
            kt_ps = tp_p.tile([P, C], bf16, tag=tp_tag)
            for o in range(CT):
                nc.tensor.transpose(
                    kt_ps[:, ts(o, P)], kn[:, o, ts(j4, P)], ident_bf[:])
            qt = tsb_pool.tile([P, C], bf16, tag="qt")
            nc.vector.tensor_copy(qt[:], qt_ps[:])
            kt = tsb_pool.tile([P, C], bf16, tag="kt")
            nc.scalar.copy(kt[:], kt_ps[:])
            pend.append((qt, kt, i))
            # Lag the sim matmuls TWO groups behind their transposes: the
            # PSUM->SBUF copy chain is ~1.6 us (sem + engine + sem), and one
            # group of PE work (~1.5 us) does not quite cover it — measured
            # 982 ns PE stall per group with lag 1.
            if len(pend) > 2:
                mm(pend.pop(0))

    if not do_a:
        return None
    for p in pend:
        mm(p)

    # ---- phase B: row softmax along the free axis ----
    e_tiles = []
    for dt in range(CT):
        neg_mx = small_pool.tile([P, 1], f32, tag="nmx")
        nc.vector.reduce_max(neg_mx[:], sim_ps[dt][:], axis=X, negate=True)
        nmx_s = small_pool.tile([P, 1], f32, tag="nmxs")
        nc.vector.tensor_scalar_mul(nmx_s[:], neg_mx[:], SCALE)
        e32 = tsb_pool.tile([P, C], f32, tag="e32")
        ssum = small_pool.tile([P, 1], f32, tag="ssum")
        nc.scalar.activation(
            e32[:], sim_ps[dt][:], EXP,
            bias=nmx_s[:], scale=SCALE, accum_out=ssum[:])
        rr = small_pool.tile([P, 1], f32, tag="rr")
        nc.vector.reciprocal(rr[:], ssum[:])
        e_sb = e_pool.tile([P, C], f32r, tag="e")
        nc.vector.tensor_scalar_mul(e_sb[:], e32[:], rr[:])
        e_tiles.append(e_sb)
    return e_tiles


def _build(reps=1, out_bf16=True):
    import concourse.bass as bass
    import concourse.tile as tile
    from concourse import bacc, mybir
    from concourse.masks import make_identity

    f32 = mybir.dt.float32
    out_dt = mybir.dt.bfloat16 if out_bf16 else f32

    nc = bacc.Bacc("TRN2", target_bir_lowering=False, debug=False,
                   num_devices=N_CORES)
    q_d = nc.dram_tensor("query", [B, C, S], f32, kind="ExternalInput")
    k_d = nc.dram_tensor("key", [B, C, S], f32, kind="ExternalInput")
    v_d = nc.dram_tensor("value", [B, C, S], f32, kind="ExternalInput")
    o_d = nc.dram_tensor("out", [B, C, S], out_dt, kind="ExternalOutput")
    dram = (q_d, k_d, v_d, o_d, out_dt)

    with tile.TileContext(nc) as tc:
        with (
            tc.tile_pool(name="const", bufs=1) as const_pool,
            tc.tile_pool(name="nat", bufs=3) as nat_pool,
            tc.tile_pool(name="tsb", bufs=4) as tsb_pool,
            tc.tile_pool(name="vpool", bufs=4) as v_pool,
            tc.tile_pool(name="epool", bufs=2 * CT) as e_pool,
            tc.tile_pool(name="small", bufs=8) as small_pool,
            tc.tile_pool(name="outp", bufs=4) as out_pool,
            tc.tile_pool(name="tp_ps", bufs=2, space="PSUM") as tp_psum,
            tc.tile_pool(name="sim_ps", bufs=CT, space="PSUM") as sim_psum,
            tc.tile_pool(name="ctx_ps", bufs=2, space="PSUM") as ctx_psum,
        ):
            ident32 = const_pool.tile([P, P], f32)
            make_identity(nc, ident32)
            ident = const_pool.tile([P, P], mybir.dt.float32r)
            nc.vector.tensor_copy(ident[:], ident32[:])
            ident_bf = const_pool.tile([P, P], mybir.dt.bfloat16)
            nc.vector.tensor_copy(ident_bf[:], ident32[:])

            pools = (nat_pool, tsb_pool, v_pool, e_pool, small_pool,
                     out_pool, tp_psum, sim_psum, ctx_psum)
            batches = [(rep, b) for rep in range(reps) for b in range(B)]
            prev = None
            for cur in batches:
                e_tiles = _emit_step(nc, pools, (ident, ident_bf), dram, cur, prev)
                prev = (cur[0], cur[1], e_tiles)
            _emit_step(nc, pools, (ident, ident_bf), dram, None, prev)

    nc.compile()
    return nc


def _get_nc(reps=1, out_bf16=True):
    key = (reps, out_bf16)
    if key not in _CACHE:
        _CACHE[key] = _build(reps, out_bf16)
    return _CACHE[key]


def run_sharded(inputs, trace=False, reps=1, out_bf16=True, **kwargs):
    """Run the SPMD kernel: returns (full_output_fp32, BassKernelResults)."""
    from concourse.bass_utils import run_bass_kernel_spmd

    nc = _get_nc(reps, out_bf16)
    in_maps = []
    for i in range(N_CORES):
        sl = slice(i * B, (i + 1) * B)
        in_maps.append({
            "query": np.ascontiguousarray(inputs["query"][sl]),
            "key": np.ascontiguousarray(inputs["key"][sl]),
            "value": np.ascontiguousarray(inputs["value"][sl]),
        })
    res = run_bass_kernel_spmd(
        nc, in_maps, core_ids=list(range(N_CORES)), trace=trace, **kwargs)
    out = np.concatenate(
        [np.asarray(res.results[i]["out"]).astype(np.float32)
         for i in range(N_CORES)], axis=0)
    return out, res


def kernel(**inputs):
    inputs = {k: np.asarray(v, dtype=np.float32) for k, v in inputs.items()}
    out, _ = run_sharded(inputs, trace=False)
    return out
